# revision 73
# baseline (speedup 1.0000x reference)
"""Distributed Trainium2 (8 NeuronCores) kernel for nn_AdaptiveAttention.

Reference computation (b=2, n=2048, d=1024, 16 heads x 64):
    qkv = x @ W_qkv; q,k,v = split(qkv)
    attn = softmax(mask(q k^T / sqrt(dh)))
    out  = (attn @ v) @ W_out + b_out

Sharding: core c in [0,8) handles batch b = c//4 and head group g = c%4
(heads 4g..4g+3).  Data parallel over b, tensor parallel over heads.
The per-head attention outputs plus softmax denominators are
AllGather'd in four chunks (bf16, all 8 ranks, overlapped with
compute); each core then computes the output projection for its
quarter of the sequence, selected with partition-id-derived dynamic
slices.

Numerics: all matmuls run bf16 operands with fp32 PSUM accumulation.
Softmax runs without max-subtraction (scores are O(1) by construction)
as exp(s) * mask, with row sums obtained by augmenting v with a ones
column; normalization happens after the AllGather via a reciprocal and
a small expansion matmul.
"""

import numpy as np
import ml_dtypes

import concourse.bass as bass
import concourse.tile as tile
from concourse import bacc, mybir
from concourse import bass_utils

BF16 = ml_dtypes.bfloat16

B = 2
N = 2048
D = 1024
HEADS = 16
HD = 64  # head dim
SCALE = HD ** -0.5
N_CORES = 8
HPC = 4  # heads per core
IB = 1024  # i-block size in attention inner tiles
NJ = N // 128  # 16 j-chunks
WARM_MMS = 1  # dummy zero-matmuls per attention iteration (PE clock keep-warm)

_cached_nc = None
_last_in_maps = None
_last_res = None
DEBUG = False


def _build():
    nc = bacc.Bacc("TRN2", target_bir_lowering=False, debug=False,
                   num_devices=N_CORES)

    f32 = mybir.dt.float32
    bf = mybir.dt.bfloat16

    xt = nc.dram_tensor("xt", [D, N], bf, kind="ExternalInput")
    wqkv = nc.dram_tensor("wqkv", [D, 768], bf, kind="ExternalInput")
    maskt = nc.dram_tensor("maskt", [N, N], bf, kind="ExternalInput")
    wout = nc.dram_tensor("wout", [D, D], bf, kind="ExternalInput")
    emat = nc.dram_tensor("emat", [2 * 8, D], bf, kind="ExternalInput")
    out = nc.dram_tensor("out", [N // 4, D], bf, kind="ExternalOutput")
    if DEBUG:
        dbg_sums = nc.dram_tensor("dbg_sums", [HEADS, 512], bf,
                                  kind="ExternalOutput")
        dbg_attnn = nc.dram_tensor("dbg_attnn", [128, 8 * 512], bf,
                                   kind="ExternalOutput")

    with tile.TileContext(nc) as tc:
        with (
            tc.tile_pool(name="res", bufs=1) as res,
            tc.tile_pool(name="dram", bufs=1, space="DRAM") as dram,
            tc.tile_pool(name="pe", bufs=6) as pe_pool,
            tc.tile_pool(name="pao", bufs=2) as pao,
            tc.tile_pool(name="ppm", bufs=3, space="PSUM") as ppm,
            tc.tile_pool(name="pp_a", bufs=1, space="PSUM") as pp_a,
        ):
            # resident tensors
            # qkt: [qT01 | qT23 | kT01 | kT23], each [128, 2048] bf16
            qkt = res.tile([128, 4 * N], bf)
            # v_aug: per j-chunk jc block of 260 cols: 4x(64 v cols + ones)
            v_aug = res.tile([128, NJ * 260], bf)
            # mask, resident (DMA'd during phase 0)
            mt = res.tile([128, NJ * N], bf)
            wout_sb = res.tile([128, 8 * D], bf)
            e8_sb = [res.tile([8, D], bf, name=f"e8_{pr}") for pr in range(2)]
            z65 = res.tile([128, 65], bf)  # zero lhsT for warm-keeper mms

            # AllGather bounce buffers, chunked by c2 = 2*pair + ib2.
            # Separate tiles per chunk so downstream dynamic-offset reads
            # depend only on their own chunk's collective.
            # ag chunk: 130 rows (2 heads x 64 attn + 2 sums), 1024 cols.
            ag_ins = [dram.tile([130, IB], bf, name=f"ag_in{c}")
                      for c in range(4)]
            ag_outs = [dram.tile([8 * 130, IB], bf, name=f"ag_out{c}",
                                 addr_space="Shared")
                       for c in range(4)]

            nc.vector.memset(z65[:], 0.0)
            nc.vector.memset(v_aug[:], 1.0)

            # ---------------- phase 0: load + projections ----------------
            with (
                tc.tile_pool(name="ph0", bufs=1) as p0,
            ):
                xtr = p0.tile([128, 8 * N], bf)
                wr = p0.tile([128, 8 * 768], bf)
                # bulk loads on the gpsimd (SWDGE) queue so they never
                # head-of-line-block the latency-critical sync-queue DMAs;
                # xt/wqkv first (projections gate on them), then mask/wout
                for k in range(8):
                    nc.gpsimd.dma_start(xtr[:, N * k:N * (k + 1)],
                                        xt[128 * k:128 * (k + 1), :])
                    nc.gpsimd.dma_start(wr[:, 768 * k:768 * (k + 1)],
                                        wqkv[128 * k:128 * (k + 1), :])
                for jc in range(NJ):
                    nc.gpsimd.dma_start(mt[:, N * jc:N * (jc + 1)],
                                        maskt[128 * jc:128 * (jc + 1), :])
                for k in range(8):
                    nc.gpsimd.dma_start(wout_sb[:, D * k:D * (k + 1)],
                                        wout[128 * k:128 * (k + 1), :])
                nc.gpsimd.dma_start(e8_sb[0][:], emat[0:8, :])
                nc.gpsimd.dma_start(e8_sb[1][:], emat[8:16, :])

                def proj_qk_group(t_i, nb):
                    wcol = 128 * t_i
                    ps = ppm.tile([128, 512], f32, name="ps_qk", tag="mm")
                    for k in range(8):
                        nc.tensor.matmul(
                            ps[:],
                            wr[:, 768 * k + wcol:768 * k + wcol + 128],
                            xtr[:, N * k + 512 * nb:N * k + 512 * nb + 512],
                            start=(k == 0), stop=(k == 7),
                        )
                    nc.vector.tensor_copy(
                        qkt[:, N * t_i + 512 * nb:N * t_i + 512 * nb + 512],
                        ps[:])

                def proj_v_group(jc):
                    ps = ppm.tile([128, 256], f32, name="ps_v", tag="mm")
                    for k in range(8):
                        nc.tensor.matmul(
                            ps[:],
                            xtr[:, N * k + 128 * jc:N * k + 128 * jc + 128],
                            wr[:, 768 * k + 512:768 * k + 768],
                            start=(k == 0), stop=(k == 7),
                        )
                    for h in range(4):
                        nc.vector.tensor_copy(
                            v_aug[:, 260 * jc + 65 * h:260 * jc + 65 * h + 64],
                            ps[:, 64 * h:64 * h + 64])

                def round_iter(pair, ib2, hh, jc, acc):
                    q_off = N * pair
                    k_off = N * (2 + pair)
                    hl = 2 * pair + hh
                    s_ps = ppm.tile([128, IB], f32, name="s_ps", tag="mm")
                    for ih in range(2):
                        nc.tensor.matmul(
                            s_ps[:, 512 * ih:512 * ih + 512],
                            qkt[64 * hh:64 * hh + 64,
                                k_off + 128 * jc:k_off + 128 * jc + 128],
                            qkt[64 * hh:64 * hh + 64,
                                q_off + IB * ib2 + 512 * ih:
                                q_off + IB * ib2 + 512 * ih + 512],
                            start=True, stop=True,
                        )
                    e_t = pe_pool.tile([128, IB], bf, name="e_t", tag="e_t")
                    nc.scalar.activation(
                        e_t[:], s_ps[:], mybir.ActivationFunctionType.Exp)
                    p_t = pe_pool.tile([128, IB], bf, name="p_t", tag="p_t")
                    nc.vector.tensor_mul(
                        p_t[:], e_t[:],
                        mt[:, N * jc + IB * ib2:N * jc + IB * ib2 + IB])
                    for ih in range(2):
                        nc.tensor.matmul(
                            acc[:, 512 * ih:512 * ih + 512],
                            v_aug[:, 260 * jc + 65 * hl:
                                  260 * jc + 65 * hl + 65],
                            p_t[:, 512 * ih:512 * ih + 512],
                            start=(jc == 0), stop=(jc == NJ - 1),
                        )
                    if jc != 0 and jc != NJ - 1:
                        for _ in range(WARM_MMS):
                            nc.tensor.matmul(
                                acc[:, 0:256], z65[:], v_aug[:, 0:256],
                                start=False, stop=False,
                                skip_group_check=True,
                            )

                def round_tail(pair, ib2, hh, acc):
                    c2 = 2 * pair + ib2
                    ao = pao.tile([65, IB], bf, name="ao", tag="ao")
                    nc.vector.tensor_copy(ao[:], acc[:])
                    nc.sync.dma_start(
                        ag_ins[c2][64 * hh:64 * hh + 64, :], ao[0:64, :])
                    nc.sync.dma_start(
                        ag_ins[c2][128 + hh:129 + hh, :], ao[64:65, :])
                    if hh == 1:
                        nc.gpsimd.collective_compute(
                            "AllGather",
                            mybir.AluOpType.bypass,
                            replica_groups=[[0, 1, 2, 3, 4, 5, 6, 7]],
                            ins=[ag_ins[c2][:].opt()],
                            outs=[ag_outs[c2][:].opt()],
                        )

                def new_acc():
                    return pp_a.tile([65, IB], f32, name="acc", tag="acc")

                # interleaved schedule: pair-0 q/k first, then the v and
                # pair-1 projections woven into the first attention rounds
                # so ACT starts early and PE never drains
                for nb in range(4):
                    proj_qk_group(0, nb)
                for nb in range(4):
                    proj_qk_group(2, nb)
                acc = new_acc()
                for jc in range(NJ):
                    proj_v_group(jc)
                    round_iter(0, 0, 0, jc, acc)
                round_tail(0, 0, 0, acc)
                acc = new_acc()
                for jc in range(NJ):
                    if jc < 4:
                        proj_qk_group(1, jc)   # qT23
                    round_iter(0, 0, 1, jc, acc)
                round_tail(0, 0, 1, acc)
                acc = new_acc()
                for jc in range(NJ):
                    if jc < 4:
                        proj_qk_group(3, jc)   # kT23
                    round_iter(0, 1, 0, jc, acc)
                round_tail(0, 1, 0, acc)

            # remaining rounds need no phase-0 tensors
            with (
                tc.tile_pool(name="ph2", bufs=1) as p2,
                tc.tile_pool(name="ost", bufs=3) as po,
            ):
                for (pair, ib2, hh) in ((0, 1, 1), (1, 0, 0), (1, 0, 1),
                                        (1, 1, 0), (1, 1, 1)):
                    acc = new_acc()
                    for jc in range(NJ):
                        round_iter(pair, ib2, hh, jc, acc)
                    round_tail(pair, ib2, hh, acc)

                # ---------- phase 2: normalize + output projection ----------
                # Each core owns i rows [256g, 256g+256) of BOTH 1024-wide
                # ib2 halves, so every chunk read is tile-static; only the
                # offsets within a chunk are dynamic.
                pid = nc.sync.partition_id()
                i0c = (pid % 4) * 256          # col offset within each chunk
                goff2 = (pid // 4) * 520       # my batch group's rank-block

                # sums rows grouped by pair: s = 2*r + hh within each pair
                # tile, so pair-0 normalization never waits on pair-1 AGs
                sums_t = [p2.tile([8, 512], bf, name=f"sums{pr}")
                          for pr in range(2)]
                recs = [p2.tile([8, 512], bf, name=f"rec{pr}")
                        for pr in range(2)]
                for pr in range(2):
                    for ib2 in range(2):
                        for r_i in range(4):
                            nc.sync.dma_start(
                                sums_t[pr][2 * r_i:2 * r_i + 2,
                                           256 * ib2:256 * ib2 + 256],
                                ag_outs[2 * pr + ib2][
                                    bass.ds(goff2 + 130 * r_i + 128, 2),
                                    bass.ds(i0c, 256)])
                    for ib2 in range(2):
                        with nc.allow_low_precision(
                                reason="softmax recip bf16"):
                            nc.vector.reciprocal(
                                recs[pr][:, 256 * ib2:256 * ib2 + 256],
                                sums_t[pr][:, 256 * ib2:256 * ib2 + 256])
                if DEBUG:
                    nc.sync.dma_start(dbg_sums[0:8, :], sums_t[0][:])
                    nc.sync.dma_start(dbg_sums[8:16, :], sums_t[1][:])

                if True:
                    attn_raw = p2.tile([128, 8 * 512], bf)
                    attn_n = p2.tile([128, 8 * 512], bf)
                    for m in (0, 2, 4, 6, 1, 3, 5, 7):  # pair-0 chunks first
                        pr = m % 2
                        for ib2 in range(2):
                            nc.sync.dma_start(
                                attn_raw[:, 512 * m + 256 * ib2:
                                         512 * m + 256 * ib2 + 256],
                                ag_outs[2 * pr + ib2][
                                    bass.ds(goff2 + 130 * (m // 2), 128),
                                    bass.ds(i0c, 256)])
                        bc = ppm.tile([128, 512], f32, name="bc", tag="mm")
                        for ib2 in range(2):
                            nc.tensor.matmul(
                                bc[:, 256 * ib2:256 * ib2 + 256],
                                e8_sb[pr][:, 128 * m:128 * m + 128],
                                recs[pr][:, 256 * ib2:256 * ib2 + 256],
                                start=True, stop=True)
                            nc.vector.tensor_mul(
                                attn_n[:, 512 * m + 256 * ib2:
                                       512 * m + 256 * ib2 + 256],
                                attn_raw[:, 512 * m + 256 * ib2:
                                         512 * m + 256 * ib2 + 256],
                                bc[:, 256 * ib2:256 * ib2 + 256])
                    if DEBUG:
                        nc.sync.dma_start(dbg_attnn[:], attn_n[:])

                    for mo in range(4):  # my-i chunks of 128
                        for nh in range(2):  # dout halves of 512
                            ps = ppm.tile([128, 512], f32, name="ps_o",
                                           tag="mm")
                            for ki, k in enumerate((0, 2, 4, 6, 1, 3, 5, 7)):
                                nc.tensor.matmul(
                                    ps[:],
                                    attn_n[:, 512 * k + 128 * mo:
                                           512 * k + 128 * mo + 128],
                                    wout_sb[:, D * k + 512 * nh:
                                            D * k + 512 * nh + 512],
                                    start=(ki == 0), stop=(ki == 7),
                                )
                            ot = po.tile([128, 512], bf, name="ot", tag="ot")
                            nc.vector.tensor_copy(ot[:], ps[:])
                            nc.sync.dma_start(
                                out[128 * mo:128 * mo + 128,
                                    512 * nh:512 * nh + 512],
                                ot[:])

    nc.compile()
    return nc


def _get_nc():
    global _cached_nc
    if _cached_nc is None:
        _cached_nc = _build()
    return _cached_nc


def kernel(x, mask, W_qkv, W_out, b_out):
    x = np.asarray(x, dtype=np.float32)
    mask = np.asarray(mask)
    W_qkv = np.asarray(W_qkv, dtype=np.float32)
    W_out = np.asarray(W_out, dtype=np.float32)
    b_out = np.asarray(b_out, dtype=np.float32)

    nc = _get_nc()

    maskt_bf = np.ascontiguousarray(mask.reshape(N, N).T).astype(BF16)
    wout_bf = W_out.astype(BF16)
    # per-pair expansion matrices: e8[pr][s, 64h+d] = 1 iff
    # h == 4*(s//2) + 2*pr + (s%2)
    emat = np.zeros((16, D), dtype=np.float32)
    for pr in range(2):
        for s in range(8):
            h = 4 * (s // 2) + 2 * pr + (s % 2)
            emat[8 * pr + s, 64 * h:64 * h + 64] = 1.0
    emat = np.ascontiguousarray(emat).astype(BF16)

    in_maps = []
    for c in range(N_CORES):
        b = c // 4
        g = c % 4
        hs = slice(g * HPC * HD, (g + 1) * HPC * HD)  # 256 cols of this core
        wq = W_qkv[:, 0 * D:1 * D][:, hs] * np.float32(SCALE)
        wk = W_qkv[:, 1 * D:2 * D][:, hs]
        wv = W_qkv[:, 2 * D:3 * D][:, hs]
        wqkv_c = np.ascontiguousarray(
            np.concatenate([wq, wk, wv], axis=1)).astype(BF16)
        xt_c = np.ascontiguousarray(x[b].T).astype(BF16)
        in_maps.append({
            "xt": xt_c,
            "wqkv": wqkv_c,
            "maskt": maskt_bf,
            "wout": wout_bf,
            "emat": emat,
        })

    global _last_in_maps, _last_res
    _last_in_maps = in_maps

    res = bass_utils.run_bass_kernel_spmd(
        nc, in_maps, core_ids=list(range(N_CORES)))
    _last_res = res

    out_full = np.empty((B, N, D), dtype=np.float32)
    for c in range(N_CORES):
        b = c // 4
        g = c % 4
        core_out = res.results[c]["out"].astype(np.float32)
        out_full[b, 256 * g:256 * g + 256, :] = core_out[0:256]
        out_full[b, 1024 + 256 * g:1024 + 256 * g + 256, :] = core_out[256:512]
    out_full += b_out
    return out_full
